# revision 3
# baseline (speedup 1.0000x reference)
"""EvolveGNN-O Trainium2 kernel (8 NeuronCores, SPMD).

Strategy (node-sharded by destination):
- Shard destination nodes across 8 cores (12500 each); each core owns all edges
  whose col (destination) falls in its range.
- Per core: edges sorted by col, grouped into 98 windows of 128 destination
  nodes; within a window grouped into 4 source-row chunks of 25000 rows so the
  int16 dma_gather indices stay in range.
- GRU + weight generation computed redundantly on every core (tiny).
- deg histogram via onehot(col) matmuls against a ones vector (PE, f32 exact).
- y2 = dinv * (x @ W^T) built per-shard, AllGather'd into a full 100k-row
  table in DRAM.
- Aggregation: dma_gather y2[row] messages per window, onehot(col) built on
  DVE via broadcast is_equal, accumulated into PSUM by TensorE matmuls.
- out = dinv * (agg + y2_local) + bias, written per-shard; host concatenates.
"""

import numpy as np

import concourse.bass as bass
import concourse.bacc as bacc
import concourse.mybir as mybir
import concourse.tile as tile
from concourse.bass_utils import run_bass_kernel_spmd
from concourse.masks import make_identity

dt = mybir.dt

N_NODES = 100000
N_EDGES = 1600000
CH = 64
NCORES = 8
NLOC = N_NODES // NCORES          # 12500 dst nodes per core
WSZ = 128                         # dst window size
W = (NLOC + WSZ - 1) // WSZ       # 98 windows (last partial: 84)
NPAD = W * WSZ                    # 12544
LAST_W = NLOC - (W - 1) * WSZ     # 84
NCHUNKS = 4
CHUNK = 25000                     # source-row chunk (int16-safe)

_BUILD_CACHE: dict = {}


def _build(Ks: tuple) -> "bacc.Bacc":
    """Build the SPMD program. Ks[g] = padded slot count per (window, chunk g),
    each a multiple of 128. Identical structure on all cores."""
    S = sum(Ks)                   # slots per window
    TPW = S // 128                # tiles per window
    TT = W * TPW                  # total tiles
    IDXC = W * (S // 16)          # idx columns
    toff = [0]
    for g in range(NCHUNKS):
        toff.append(toff[-1] + Ks[g] // 128)

    nc = bacc.Bacc("TRN2", target_bir_lowering=False, debug=False,
                   num_devices=NCORES)

    # ---- inputs (per core) ----
    x_sh = nc.dram_tensor("x_sh", [128, W * CH], dt.float32, kind="ExternalInput")
    colrel = nc.dram_tensor("colrel", [128, TT], dt.float32, kind="ExternalInput")
    idx_in = nc.dram_tensor("idx_in", [128, IDXC], dt.int16, kind="ExternalInput")
    counts = nc.dram_tensor("counts", [1, W * NCHUNKS], dt.int32, kind="ExternalInput")
    mw_in = nc.dram_tensor("mw_in", [64], dt.float32, kind="ExternalInput")
    wih_in = nc.dram_tensor("wih_in", [128, 2 * CH], dt.float32, kind="ExternalInput")
    bih_in = nc.dram_tensor("bih_in", [192], dt.float32, kind="ExternalInput")
    bhh_in = nc.dram_tensor("bhh_in", [192], dt.float32, kind="ExternalInput")
    wtw_in = nc.dram_tensor("wtw_in", [128, 32 * CH], dt.float32, kind="ExternalInput")
    wtb_in = nc.dram_tensor("wtb_in", [4096], dt.float32, kind="ExternalInput")
    gbias_in = nc.dram_tensor("gbias_in", [64], dt.float32, kind="ExternalInput")

    # ---- outputs ----
    out_d = nc.dram_tensor("out_d", [128, W * CH], dt.float32, kind="ExternalOutput")
    dbg = bool(int(__import__("os").environ.get("GNN_DEBUG", "0")))
    if dbg:
        deg_o = nc.dram_tensor("deg_o", [128, W], dt.float32, kind="ExternalOutput")
        dinv_o = nc.dram_tensor("dinv_o", [128, W], dt.float32, kind="ExternalOutput")
        wt_o = nc.dram_tensor("wt_o", [64, 64], dt.float32, kind="ExternalOutput")
        y2s_o = nc.dram_tensor("y2s_o", [128, W * CH], dt.float32, kind="ExternalOutput")
        y2f_o = nc.dram_tensor("y2f_o", [N_NODES, CH], dt.float32, kind="ExternalOutput")

    # ---- internal DRAM ----
    y2_shard = nc.dram_tensor("y2_shard", [NLOC, CH], dt.float32)
    y2_full = nc.dram_tensor("y2_full", [N_NODES, CH], dt.float32, addr_space="Shared")

    with tile.TileContext(nc) as tc:
        with (
            tc.tile_pool(name="res", bufs=1) as res,
            tc.tile_pool(name="work", bufs=2) as work,
            tc.tile_pool(name="msgsp", bufs=3) as msgsp,
        ):
            # ================= resident loads =================
            x_sb = res.tile([128, W, CH], dt.float32)
            nc.sync.dma_start(x_sb[:], x_sh[:].rearrange("p (w c) -> p w c", c=CH))
            col_sb = res.tile([128, TT], dt.float32)
            nc.sync.dma_start(col_sb[:], colrel[:])
            idx_sb = res.tile([128, IDXC], dt.int16)
            nc.sync.dma_start(idx_sb[:], idx_in[:])
            counts_sb = res.tile([1, W * NCHUNKS], dt.int32)
            nc.sync.dma_start(counts_sb[:], counts[:])
            bias_sb = res.tile([128, CH], dt.float32)
            nc.sync.dma_start(bias_sb[:], gbias_in[None, :].to_broadcast([128, CH]))
            iota_sb = res.tile([128, 128], dt.float32)
            nc.gpsimd.iota(iota_sb[:], pattern=[[1, 128]], base=0,
                           channel_multiplier=0,
                           allow_small_or_imprecise_dtypes=True)
            ident = res.tile([128, 128], dt.float32)
            make_identity(nc, ident[:])
            ones_sb = res.tile([128, 1], dt.float32)
            nc.vector.memset(ones_sb[:], 1.0)

            y2_sb = res.tile([128, W, CH], dt.float32)
            out_sb = res.tile([128, W, CH], dt.float32)
            deg_sb = res.tile([128, W], dt.float32)
            dinv_sb = res.tile([128, W], dt.float32)
            WT_sb = res.tile([64, 64], dt.float32)

            # ================= phase A: W generation =================
            with tc.tile_pool(name="psA", bufs=2, space="PSUM") as psA:
                wih_sb = work.tile([128, 2, CH], dt.float32, tag="wih")
                nc.sync.dma_start(wih_sb[:], wih_in[:].rearrange("p (t c) -> p t c", c=CH))
                wihT_sb = work.tile([64, 256], dt.float32, tag="wihT")
                for t in range(2):
                    trp = psA.tile([64, 128], dt.float32, space="PSUM", tag="tr")
                    nc.tensor.transpose(trp[:], wih_sb[:, t, :], ident[:])
                    nc.vector.tensor_copy(wihT_sb[:, 128 * t:128 * (t + 1)], trp[:])

                mw_sb = work.tile([64, 1], dt.float32, tag="mw")
                nc.sync.dma_start(mw_sb[:], mw_in[:, None])
                bih_sb = work.tile([64, 3], dt.float32, tag="bih")
                nc.sync.dma_start(bih_sb[:], bih_in[:].rearrange("(s p) -> p s", p=64))
                bhh_sb = work.tile([64, 3], dt.float32, tag="bhh")
                nc.sync.dma_start(bhh_sb[:], bhh_in[:].rearrange("(s p) -> p s", p=64))

                gi_sb = work.tile([64, 3], dt.float32, tag="gi")
                for s in range(3):
                    gps = psA.tile([64, 1], dt.float32, space="PSUM", tag="gi")
                    nc.tensor.matmul(gps[:], wihT_sb[:, 64 * s:64 * (s + 1)],
                                     mw_sb[:], start=True, stop=True)
                    nc.vector.tensor_copy(gi_sb[:, s:s + 1], gps[:])

                bsum = work.tile([64, 2], dt.float32, tag="bsum")
                nc.vector.tensor_add(bsum[:], bih_sb[:, 0:2], bhh_sb[:, 0:2])
                gates = work.tile([64, 4], dt.float32, tag="gates")  # r z n um
                nc.scalar.activation(gates[:, 0:1], gi_sb[:, 0:1],
                                     mybir.ActivationFunctionType.Sigmoid,
                                     bias=bsum[:, 0:1])
                nc.scalar.activation(gates[:, 1:2], gi_sb[:, 1:2],
                                     mybir.ActivationFunctionType.Sigmoid,
                                     bias=bsum[:, 1:2])
                nb = work.tile([64, 1], dt.float32, tag="nb")
                nc.vector.tensor_mul(nb[:], gates[:, 0:1], bhh_sb[:, 2:3])
                nc.vector.tensor_add(nb[:], nb[:], bih_sb[:, 2:3])
                nc.scalar.activation(gates[:, 2:3], gi_sb[:, 2:3],
                                     mybir.ActivationFunctionType.Tanh, bias=nb[:])
                omz = work.tile([64, 1], dt.float32, tag="omz")
                nc.vector.tensor_scalar(omz[:], gates[:, 1:2], -1.0, 1.0,
                                        mybir.AluOpType.mult, mybir.AluOpType.add)
                um_sb = work.tile([64, 1], dt.float32, tag="um")
                nc.vector.tensor_mul(um_sb[:], omz[:], gates[:, 2:3])

                wtw_sb = work.tile([128, 32, CH], dt.float32, tag="wtw")
                nc.sync.dma_start(wtw_sb[:], wtw_in[:].rearrange("p (t c) -> p t c", c=CH))
                wtbT_sb = work.tile([64, 64], dt.float32, tag="wtbT")
                nc.sync.dma_start(wtbT_sb[:], wtb_in[:].rearrange("(o p) -> p o", p=64))
                W_ps = psA.tile([64, 64], dt.float32, space="PSUM", tag="W")
                for t in range(32):
                    trp = psA.tile([64, 128], dt.float32, space="PSUM", tag="tr")
                    nc.tensor.transpose(trp[:], wtw_sb[:, t, :], ident[:])
                    trs = work.tile([64, 128], dt.float32, tag="trs")
                    nc.vector.tensor_copy(trs[:], trp[:])
                    for b in range(2):
                        nc.tensor.matmul(W_ps[:, 2 * t + b:2 * t + b + 1],
                                         trs[:, 64 * b:64 * (b + 1)], um_sb[:],
                                         start=True, stop=True,
                                         skip_group_check=True)
                nc.vector.tensor_add(WT_sb[:], W_ps[:], wtbT_sb[:])

                # ================= phase A2: deg histogram =================
                for w in range(W):
                    oh = work.tile([128, TPW, 128], dt.float32, tag="degoh")
                    nc.vector.tensor_tensor(
                        out=oh[:],
                        in0=col_sb[:, w * TPW:(w + 1) * TPW].unsqueeze(2)
                            .to_broadcast([128, TPW, 128]),
                        in1=iota_sb[:].unsqueeze(1).to_broadcast([128, TPW, 128]),
                        op=mybir.AluOpType.is_equal)
                    dps = psA.tile([128, 1], dt.float32, space="PSUM", tag="deg")
                    for t in range(TPW):
                        nc.tensor.matmul(dps[:], oh[:, t, :], ones_sb[:],
                                         start=(t == 0), stop=(t == TPW - 1))
                    nc.vector.tensor_copy(deg_sb[:, w:w + 1], dps[:])

                # dinv = 1/sqrt(deg+1)
                sq = work.tile([128, W], dt.float32, tag="sq")
                nc.scalar.activation(sq[:], deg_sb[:],
                                     mybir.ActivationFunctionType.Sqrt, bias=1.0)
                nc.vector.reciprocal(dinv_sb[:], sq[:])

            # ================= phase B: y2 = dinv * (x @ W^T) =================
            with tc.tile_pool(name="psB", bufs=2, space="PSUM") as psB:
                for w in range(W):
                    xTp = psB.tile([64, 128], dt.float32, space="PSUM", tag="xT")
                    nc.tensor.transpose(xTp[:], x_sb[:, w, :], ident[:])
                    xTs = work.tile([64, 128], dt.float32, tag="xTs")
                    nc.vector.tensor_copy(xTs[:], xTp[:])
                    xwp = psB.tile([128, CH], dt.float32, space="PSUM", tag="xw")
                    nc.tensor.matmul(xwp[:], xTs[:], WT_sb[:], start=True, stop=True)
                    nc.vector.tensor_scalar_mul(y2_sb[:, w, :], xwp[:],
                                                dinv_sb[:, w:w + 1])

            # y2 shard -> DRAM (node-major), then AllGather
            nc.sync.dma_start(
                y2_shard[0:(W - 1) * WSZ, :].rearrange("(w p) c -> p w c", p=128),
                y2_sb[:, 0:W - 1, :])
            nc.sync.dma_start(y2_shard[(W - 1) * WSZ:NLOC, :],
                              y2_sb[0:LAST_W, W - 1, :])
            nc.gpsimd.collective_compute(
                "AllGather", mybir.AluOpType.bypass,
                replica_groups=[list(range(NCORES))],
                ins=[y2_shard[:]], outs=[y2_full[:]])

            # ================= phase C: gather + aggregate =================
            cnt_reg = nc.gpsimd.alloc_register("cnt")
            with tc.tile_pool(name="psC", bufs=2, space="PSUM") as psC:
                # Zero the msgs slots once: gather tail-trims padded (-1)
                # indices, leaving untouched slots; uninitialized SBUF can
                # hold NaN and 0*NaN poisons the PE accumulation.
                for _ in range(3):
                    mz = msgsp.tile([128, TPW, CH], dt.float32, tag="msgs")
                    nc.vector.memset(mz[:], 0.0)
                for w in range(W):
                    msgs = msgsp.tile([128, TPW, CH], dt.float32, tag="msgs")
                    for g in range(NCHUNKS):
                        cell = w * NCHUNKS + g
                        nc.gpsimd.reg_load(cnt_reg, counts_sb[:1, cell:cell + 1])
                        nc.gpsimd.dma_gather(
                            msgs[:, toff[g]:toff[g + 1], :],
                            y2_full[g * CHUNK:(g + 1) * CHUNK, :],
                            idx_sb[:, w * (S // 16) + (sum(Ks[:g]) // 16):
                                   w * (S // 16) + (sum(Ks[:g + 1]) // 16)],
                            Ks[g], cnt_reg, CH)
                    oh = work.tile([128, TPW, 128], dt.float32, tag="aggoh")
                    nc.vector.tensor_tensor(
                        out=oh[:],
                        in0=col_sb[:, w * TPW:(w + 1) * TPW].unsqueeze(2)
                            .to_broadcast([128, TPW, 128]),
                        in1=iota_sb[:].unsqueeze(1).to_broadcast([128, TPW, 128]),
                        op=mybir.AluOpType.is_equal)
                    aps = psC.tile([128, CH], dt.float32, space="PSUM", tag="agg")
                    for t in range(TPW):
                        nc.tensor.matmul(aps[:], oh[:, t, :], msgs[:, t, :],
                                         start=(t == 0), stop=(t == TPW - 1))
                    # out = dinv*(agg + y2_local) + bias
                    nc.vector.tensor_add(out_sb[:, w, :], aps[:], y2_sb[:, w, :])
                    nc.vector.tensor_scalar_mul(out_sb[:, w, :], out_sb[:, w, :],
                                                dinv_sb[:, w:w + 1])
                    nc.vector.tensor_add(out_sb[:, w, :], out_sb[:, w, :],
                                         bias_sb[:])

            nc.sync.dma_start(out_d[:], out_sb[:].rearrange("p w c -> p (w c)"))
            if dbg:
                nc.sync.dma_start(deg_o[:], deg_sb[:])
                nc.sync.dma_start(dinv_o[:], dinv_sb[:])
                nc.sync.dma_start(wt_o[:], WT_sb[:])
                nc.sync.dma_start(y2s_o[:], y2_sb[:].rearrange("p w c -> p (w c)"))
                nc.sync.dma_start(y2f_o[:], y2_full[:])

    nc.compile()
    return nc


def _host_prep(x, edge_index, memory_weights, gru_w_ih, gru_b_ih, gru_b_hh,
               wt_w, wt_b, gcn_bias):
    rows = np.asarray(edge_index[0], dtype=np.int64)
    cols = np.asarray(edge_index[1], dtype=np.int64)
    x = np.asarray(x, dtype=np.float32)

    order = np.argsort(cols, kind="stable")
    rows_s = rows[order].astype(np.int32)
    cols_s = cols[order].astype(np.int32)
    core_bounds = np.searchsorted(cols_s, np.arange(NCORES + 1) * NLOC)

    # per-(core, window, chunk) counts
    all_cells = []
    for j in range(NCORES):
        lo, hi = core_bounds[j], core_bounds[j + 1]
        ec = cols_s[lo:hi] - j * NLOC
        er = rows_s[lo:hi]
        w = ec >> 7
        g = er // CHUNK
        cell = w * NCHUNKS + g
        cnt = np.bincount(cell, minlength=W * NCHUNKS).astype(np.int32)
        all_cells.append((ec, er, cell, cnt))
    cnt_max = np.max(np.stack([c[3] for c in all_cells]), axis=0)
    Ks = tuple(int(np.ceil(cnt_max.reshape(W, NCHUNKS)[:, g].max() / 128) * 128)
               for g in range(NCHUNKS))
    S = sum(Ks)
    TPW = S // 128
    base_off = np.zeros(W * NCHUNKS, np.int64)
    for w in range(W):
        off = w * S
        for g in range(NCHUNKS):
            base_off[w * NCHUNKS + g] = off
            off += Ks[g]

    in_maps = []
    for j in range(NCORES):
        ec, er, cell, cnt = all_cells[j]
        colrel = np.full(W * S, -1.0, np.float32)
        idxs = np.full(W * S, -1, np.int16)
        # rank within cell
        cello = np.argsort(cell, kind="stable")
        ranks = np.empty(len(cell), np.int64)
        cs = np.zeros(W * NCHUNKS + 1, np.int64)
        np.cumsum(cnt, out=cs[1:])
        ranks[cello] = np.arange(len(cell)) - cs[cell[cello]]
        slot = base_off[cell] + ranks
        colrel[slot] = (ec & 127).astype(np.float32)
        idxs[slot] = (er - (er // CHUNK) * CHUNK).astype(np.int16)

        col_tiles = colrel.reshape(W * TPW, 128).T.copy()  # [128, TT]
        # idx wrapped layout per (w,g) run, replicated over 8 gpsimd cores
        idx_cols = np.empty((16, W * S // 16), np.int16)
        cpos = 0
        for w in range(W):
            for g in range(NCHUNKS):
                K = Ks[g]
                run = idxs[w * S + (base_off[w * NCHUNKS + g] - w * S):
                           w * S + (base_off[w * NCHUNKS + g] - w * S) + K]
                idx_cols[:, cpos:cpos + K // 16] = run.reshape(K // 16, 16).T
                cpos += K // 16
        idx_rep = np.tile(idx_cols, (8, 1)).copy()

        xp = np.zeros((NPAD, CH), np.float32)
        xp[:NLOC] = x[j * NLOC:(j + 1) * NLOC]
        x_shuf = xp.reshape(W, 128, CH).transpose(1, 0, 2).reshape(128, W * CH).copy()

        wih_p = np.zeros((256, CH), np.float32)
        wih_p[:192] = np.asarray(gru_w_ih, np.float32)
        wih_shuf = wih_p.reshape(2, 128, CH).transpose(1, 0, 2).reshape(128, 2 * CH).copy()
        wtw = np.asarray(wt_w, np.float32)
        wtw_shuf = wtw.reshape(32, 128, CH).transpose(1, 0, 2).reshape(128, 32 * CH).copy()

        in_maps.append(dict(
            x_sh=x_shuf, colrel=col_tiles, idx_in=idx_rep,
            counts=cnt[None, :].astype(np.int32),
            mw_in=np.asarray(memory_weights, np.float32),
            wih_in=wih_shuf,
            bih_in=np.asarray(gru_b_ih, np.float32),
            bhh_in=np.asarray(gru_b_hh, np.float32),
            wtw_in=wtw_shuf,
            wtb_in=np.asarray(wt_b, np.float32),
            gbias_in=np.asarray(gcn_bias, np.float32),
        ))
    return Ks, in_maps


def kernel(x, edge_index, memory_weights, gru_w_ih, gru_w_hh, gru_b_ih,
           gru_b_hh, wt_w, wt_b, gcn_bias, _want_trace=False):
    Ks, in_maps = _host_prep(x, edge_index, memory_weights, gru_w_ih,
                             gru_b_ih, gru_b_hh, wt_w, wt_b, gcn_bias)
    if Ks not in _BUILD_CACHE:
        _BUILD_CACHE[Ks] = _build(Ks)
    nc = _BUILD_CACHE[Ks]
    res = run_bass_kernel_spmd(nc, in_maps, list(range(NCORES)),
                               trace=_want_trace)
    out = np.empty((N_NODES, CH), np.float32)
    for j in range(NCORES):
        o = res.results[j]["out_d"].reshape(128, W, CH).transpose(1, 0, 2)
        out[j * NLOC:(j + 1) * NLOC] = o.reshape(NPAD, CH)[:NLOC]
    kernel._last_result = res
    return out
